# revision 56
# baseline (speedup 1.0000x reference)
"""Trainium2 Bass kernel for nn_MHA_43095701848407.

MHA forward: qkv = x @ W_qkv, RoPE on q/k, causal softmax attention,
y @ W_proj.  B=4, T=2048, C=2048, 16 heads, head_dim=128.

Sharding (8 cores): tensor-parallel over heads (4 shards x 4 heads) x
data-parallel over batch (2 groups x 2 batches).  core = group*4 + shard.

Design notes (measured on hw): PE wall per matmul is ~N/2.4GHz + 3ns
with bf16 operands (weight loads fully hidden by the background weight
buffer; fp32r pays ~70ns extra per load), so the kernel minimizes
streamed PE columns and instruction count, and keeps every other engine
off the PE critical path:
- all matmul operands bf16 (abundant tolerance: gate 2e-2, measured 5e-3)
- q^T/k^T/v SBUF-resident per batch; per-batch pipeline qkv->attn->proj
  with x/W prefetch on both DMA-issue queues (sync + scalar)
- W_qkv host-relaid to [p, chunk, ko, f] for 4KB-contiguous DMA runs
- RoPE rotate-half as a partition-strided SBUF->SBUF DMA pair (off PE);
  the sign folded into sin on the host
- v computed transposed in the W-stream, moved to natural layout by one
  wide DMA-transpose per (chunk, slab); v chunks run before q/k so the
  transposes drain during qk compute
- attention flattened over (head, q-tile, key-block) with a software
  pipeline skew carried across boundaries; slab-1 runs qk chunks first
  and v chunks LAST so the rope tail is off the attention critical
  path, and the first five attention blocks pre-emit inside the v-chunk
  region (score tiles borrow qkv psum slots) so their exps hide under
  the v matmuls; single merged exp per key block from a 2-bank PSUM
  tile; causal mask via triangular-mask multiply on DVE
- the l (softmax denominator) ones-matmul halves are col-tiled to PE
  quadrants 0/1 (out partitions 0/32 of one 1-bank tile) so both
  streams run CONCURRENTLY -- halves the l cost (~29us/kernel)
- softmax tails split in two parts deferred ~3 blocks apart so nothing
  head-of-line blocks the DVE/PE queues: part 1 does one base-0
  reciprocal_approx_fast over partitions [0:33] (the op and
  partition_broadcast both misbehave on base-partition-32 APs on hw)
  plus the half-A gpsimd broadcast; part 2 broadcasts half B via a K=1
  PE matmul (ones row at partition 32, row-group 1) into the dead l
  bank; the final group's part 2 is emitted inside proj co=0 before
  the h3 matmuls
- first chunks of startup/slab-1 are FUSED ko-interleaved (2-3 chunks)
  so PE x-consumption (~200-300 GB/s) stays under DMA supply; startup
  W/x transfers graded small-first; later W chunks staggered through
  the slab-0 loop (one sync-queue issue per chunk -- bulk dumps cause
  multi-us semaphore-reuse throttling that head-of-line blocks the
  queue, and scalar-queue DMA issues must never precede time-critical
  ACT evac copies)
- avoid nc.gpsimd.dma_start anywhere near partition_broadcast: SWDGE
  forces a gpsimd library swap costing ~7us
- proj jt-outer/h-inner: consecutive matmuls accumulate into one psum
  bank and each jt's evacuation overlaps the next jt's matmuls; wp
  streamed per c-chunk, out writes split across both DMA queues
Host sums the 4 head-shard bf16 partials per batch in f32.

Self-contained: shapes/sharding hardcoded; inputs full-size numpy arrays.
"""

import math
import os
import sys
import types

import ml_dtypes
import numpy as np

import concourse.bass as bass
import concourse.mybir as mybir
import concourse.tile as tile
from concourse import bacc
from concourse.bass_utils import run_bass_kernel_spmd

F32 = mybir.dt.float32
BF16 = mybir.dt.bfloat16
AF = mybir.ActivationFunctionType
ALU = mybir.AluOpType
NPBF = ml_dtypes.bfloat16

# Problem shape (hardcoded per contract)
B, T, C = 4, 2048, 2048
H, HD = 16, 128
NCORES = 8
BGROUPS, HSHARDS = 2, 4  # batch groups x head shards
B_LOC = B // BGROUPS  # 2 batches per core
H_LOC = H // HSHARDS  # 4 heads per core
FQK = H_LOC * HD  # 512 features for q (and for k)
FV = H_LOC * HD  # 512 for v
NCH = 12  # qkv feature chunks of 128 (4 q + 4 k + 4 v)
# slab 0: v chunks (8..11) first so their DMA transposes drain during qk
# compute; qk interleaved q0,k0,... so early heads are ready first.
# slab 1: qk first / v LAST so the rope DVE/DMA tail of the final qk
# chunks is fully hidden under the v chunks (it used to stall the first
# attention drains ~6us); the first attention blocks pre-emit into the
# v-chunk region.
CHUNK_ORDER0 = [8, 9, 10, 11, 0, 4, 1, 5, 2, 6, 3, 7]
CHUNK_ORDER1 = [0, 4, 1, 5, 2, 6, 3, 7, 8, 9, 10, 11]
KO = C // 128  # 16 contraction chunks
KOG = 4  # x DMA granularity: 4 ko chunks per transfer
TSLAB = 1024
NSLAB = T // TSLAB  # 2 t-slabs per batch
QT = 1024  # attention q tile
NQT = T // QT  # 2 q tiles
NKBT = QT // 128  # 8 key blocks per q tile width
SCALE = 1.0 / math.sqrt(HD)

_CACHED = {}


def _install_ntff_hook():
    """Register the axon NTFF profile hook (container's antenv lacks it)."""
    if "antenv.axon_hooks" in sys.modules:
        return
    try:
        mod = types.ModuleType("antenv.axon_hooks")
        holder = [None]
        mod.set_axon_ntff_profile_hook = lambda h: holder.__setitem__(0, h)
        mod.get_axon_ntff_profile_hook = lambda: holder[0]
        sys.modules["antenv.axon_hooks"] = mod
        import antenv

        antenv.axon_hooks = mod
        if "/root/.axon_site" not in sys.path:
            sys.path.insert(0, "/root/.axon_site")
        from trn_agent_boot.trn_boot import _ntff_profile_via_ctypes

        mod.set_axon_ntff_profile_hook(
            _ntff_profile_via_ctypes("/opt/axon/libaxon_pjrt.so")
        )
    except Exception:
        sys.modules.pop("antenv.axon_hooks", None)


def build_nc():
    nc = bacc.Bacc("TRN2", target_bir_lowering=False, debug=False)

    x_t = nc.dram_tensor("x_t", [B_LOC, C, T], BF16, kind="ExternalInput").ap()
    w_qkv = nc.dram_tensor("w_qkv", [128, NCH, KO, 128], BF16,
                           kind="ExternalInput").ap()
    w_proj = nc.dram_tensor("w_proj", [FV, C], BF16, kind="ExternalInput").ap()
    cos_t = nc.dram_tensor("cos_t", [HD, T], BF16, kind="ExternalInput").ap()
    sin_t = nc.dram_tensor("sin_t", [HD, T], BF16, kind="ExternalInput").ap()
    ones_col = nc.dram_tensor("ones_col", [128, 1], BF16, kind="ExternalInput").ap()
    ones_bc = nc.dram_tensor("ones_bc", [33, 128], BF16, kind="ExternalInput").ap()
    tri = nc.dram_tensor("tri", [128, 128], BF16, kind="ExternalInput").ap()
    out_t = nc.dram_tensor("out_t", [B_LOC, C, T], BF16, kind="ExternalOutput").ap()

    with tile.TileContext(nc) as tc:
        with nc.allow_low_precision(reason="bf16 matmuls by design; tol 2e-2"):
            _emit(nc, tc, x_t, w_qkv, w_proj, cos_t, sin_t, ones_col,
                  ones_bc, tri, out_t)
    nc.compile()
    return nc


def _emit(nc, tc, x_t, w_qkv, w_proj, cos_t, sin_t, ones_col,
          ones_bc, tri, out_t):
    with (
        tc.tile_pool(name="consts", bufs=1) as consts,
        tc.tile_pool(name="wq", bufs=1) as wqpool,
        tc.tile_pool(name="wp", bufs=3) as wppool,
        tc.tile_pool(name="qkres", bufs=1) as qkres,
        tc.tile_pool(name="vres", bufs=1) as vres,
        tc.tile_pool(name="yres", bufs=1) as yres,
        tc.tile_pool(name="xpool", bufs=3) as xpool,
        tc.tile_pool(name="rawpool", bufs=2) as rawpool,
        tc.tile_pool(name="shufpool", bufs=2) as shufpool,
        tc.tile_pool(name="vtpool", bufs=2) as vtpool,
        tc.tile_pool(name="ppool", bufs=7) as ppool,
        tc.tile_pool(name="nfpool", bufs=1) as nfpool,
        tc.tile_pool(name="nbpool", bufs=1) as nbpool,
        tc.tile_pool(name="bcpool", bufs=2) as bcpool,
        tc.tile_pool(name="opool", bufs=5) as opool,
    ):
        sb_pools = dict(raw=rawpool, shuf=shufpool, vt=vtpool, p=ppool,
                        nf=nfpool, nb=nbpool, bc=bcpool, o=opool)
        def load_x_half(b, js, hh, eng=None, kogs=None, split=None):
            # kogs (list of ko-group sizes) splits the transfer so
            # compute can start on the first ko chunks; split=(i, eng2)
            # issues pieces i.. on a second queue
            eng = eng or nc.sync
            x3 = x_t[b].rearrange("(ko p) t -> p ko t", p=128)
            hsl = slice(js * TSLAB + hh * 512, js * TSLAB + (hh + 1) * 512)
            x_h = xpool.tile([128, KO, 512], BF16, name="x_h")
            if kogs is None:
                kogs = [KO]
            elif isinstance(kogs, int):
                kogs = [kogs] * (KO // kogs)
            k0 = 0
            for pi, kg in enumerate(kogs):
                e = eng
                if split is not None and pi >= split[0]:
                    e = split[1]
                e.dma_start(x_h[:, k0:k0 + kg, :], x3[:, k0:k0 + kg, hsl])
                k0 += kg
            return x_h

        # Startup (r2 arrangement, measured best): w8/w9 interleaved
        # ko-pieces lead the scalar queue, x00/x01 graded, w10/w0 on
        # sync, w11 on scalar, consts after; the remaining W chunks are
        # staggered through the slab-0 chunk loop on sync so neither
        # queue accumulates sem-reuse throttling.
        GRADED = [2, 2, 4, 4, 4]
        w_sb = wqpool.tile([128, NCH, KO, 128], BF16)
        for kg in range(4):
            ks = slice(kg * 4, (kg + 1) * 4)
            nc.scalar.dma_start(w_sb[:, 8, ks, :], w_qkv[:, 8, ks, :])
            nc.scalar.dma_start(w_sb[:, 9, ks, :], w_qkv[:, 9, ks, :])
        x00 = load_x_half(0, 0, 0, nc.sync, kogs=GRADED)
        # last two x01 pieces ride the sync queue: they are needed at
        # ~21us and the scalar queue's W issue slices delay them past
        # that; sync has drained its startup issues by then
        x01 = load_x_half(0, 0, 1, nc.scalar, kogs=GRADED,
                          split=(3, nc.sync))
        nc.sync.dma_start(w_sb[:, 10, :, :], w_qkv[:, 10, :, :])
        nc.scalar.dma_start(w_sb[:, 11, :, :], w_qkv[:, 11, :, :])
        nc.sync.dma_start(w_sb[:, 0, :, :], w_qkv[:, 0, :, :])
        cos_sb = consts.tile([HD, T], BF16)
        nc.scalar.dma_start(cos_sb, cos_t)
        sin_sb = consts.tile([HD, T], BF16)
        nc.scalar.dma_start(sin_sb, sin_t)
        tri_sb = consts.tile([128, 128], BF16)
        nc.sync.dma_start(tri_sb, tri)
        ones_c_sb = consts.tile([128, 1], BF16)
        nc.sync.dma_start(ones_c_sb, ones_col)
        ones_bc_sb = consts.tile([33, 128], BF16)
        nc.sync.dma_start(ones_bc_sb, ones_bc)
        w_issued = {8, 9, 10, 11, 0}

        def w_prefetch(ci):
            # at the head of slab-0 chunk ci (batch 0): issue the W
            # chunk needed ~3 chunks ahead on the sync queue
            idx = ci + 3
            if idx < NCH:
                f = CHUNK_ORDER0[idx]
                if f not in w_issued:
                    w_issued.add(f)
                    nc.sync.dma_start(w_sb[:, f, :, :], w_qkv[:, f, :, :])

        # Per-batch resident activations (reused across batches; the tile
        # framework serializes WAR hazards between batches automatically).
        qk_sb = qkres.tile([128, 8, T], BF16)  # chunks: q heads 0-3, k heads 4-7
        v_sb = vres.tile([128, T // 128, FV], BF16)  # natural [t, fv]
        y_sb = yres.tile([128, H_LOC, T], BF16)  # y^T per head

        xq = [x00, x01, load_x_half(0, 1, 0)]

        for b in range(B_LOC):
            halves = {(0, 0): xq[0], (0, 1): xq[1]}
            if xq[2] is not None:
                halves[(1, 0)] = xq[2]
            else:
                halves[(1, 0)] = load_x_half(b, 1, 0)
            pending = []

            def attn_pro(qkps_pool, pi):
                # pre-emit attention block pi's scores+exp inside the
                # slab-1 v-chunk region; score tile borrows a qkv psum
                # slot; blocks 5-7 store p in the rope scratch tiles
                # (raw/shuf), which are idle from the last slab-1 qk
                # chunk until the next batch's first rope
                def alloc_s():
                    return qkps_pool.tile([128, TSLAB], F32, name="ps")
                def p_alloc():
                    return ppool.tile([128, QT], BF16, name="p_sb")
                pending.append(_attn_block(nc, alloc_s, p_alloc, qk_sb,
                                           tri_sb, (0, 0, pi)))

            _phase_qkv(nc, tc, b, halves, load_x_half, w_sb, cos_sb, sin_sb,
                       qk_sb, v_sb, sb_pools, attn_pro,
                       w_prefetch if b == 0 else None)
            nxq = [None, None, None]
            if b + 1 < B_LOC:
                # prefetch next batch's x during this batch's attention
                nxq = [load_x_half(b + 1, 0, 0),
                       load_x_half(b + 1, 0, 1, nc.scalar),
                       load_x_half(b + 1, 1, 0)]
            left, flush = _phase_attn(nc, tc, b, qk_sb, v_sb, y_sb,
                                      ones_c_sb, ones_bc_sb, tri_sb,
                                      sb_pools, pending)
            _phase_proj(nc, tc, b, wppool, w_proj, y_sb, out_t, sb_pools,
                        left, flush)
            xq = nxq


def _phase_qkv(nc, tc, b, halves, load_x_half, w_sb, cos_sb, sin_sb,
               qk_sb, v_sb, sb_pools, attn_pro=None, w_prefetch=None):
    """qkv^T = W.T @ x^T in 128-feature chunks (per-slab CHUNK_ORDER).
    RoPE rotate-half via partition-strided SBUF->SBUF DMA:
    roped = raw*cos + shuf(raw)*sin_signed.  v chunks are evacuated as
    v^T and moved to natural [t, fv] layout with one wide DMA transpose
    per (chunk, slab)."""
    rawpool, shufpool, vtpool = (sb_pools["raw"], sb_pools["shuf"],
                                 sb_pools["vt"])
    with tc.tile_pool(name="qkps", bufs=3, space="PSUM") as qkps:
        for js in range(NSLAB):
            order = CHUNK_ORDER0 if js == 0 else CHUNK_ORDER1
            tsl = slice(js * TSLAB, (js + 1) * TSLAB)
            if js == 1 and (1, 1) not in halves:
                # slot for (1,1) frees once slab-0 compute is done
                halves[(1, 1)] = load_x_half(b, 1, 1, kogs=KOG)
            h0 = halves[(js, 0)]
            h1 = halves[(js, 1)]
            for ci, f in enumerate(order):
                if w_prefetch is not None and js == 0:
                    w_prefetch(ci)
                if attn_pro is not None and js == NSLAB - 1 and ci >= 8:
                    # pre-emit attention blocks per v chunk: their exps
                    # run on ACT while the PE chews the v chunks
                    if ci == 8:
                        attn_pro(qkps, 0)
                        attn_pro(qkps, 1)
                    else:
                        attn_pro(qkps, ci - 7)
                nfuse = 0
                if b == 0 and js == 0:
                    nfuse = 2  # startup: w10 lands later on sync
                elif js == 1:
                    nfuse = 3  # slack for the late (1,1) half
                if nfuse and ci < nfuse:
                    # fuse the first chunks: half 0 ko-interleaved so the
                    # PE consumes x at a fraction of the usual rate,
                    # under the DMA supply rate, and the late half-1
                    # (startup x01 / slab-1 (1,1)) is not needed for
                    # nfuse chunk-times; half 1 then runs per-chunk so
                    # the evacs spread out
                    if ci > 0:
                        continue  # handled by the fused ci==0 pass
                    ff = order[:nfuse]
                    pss = [qkps.tile([128, TSLAB], F32, name="ps")
                           for _ in range(nfuse)]
                    for ko in range(KO):
                        for j in range(nfuse):
                            nc.tensor.matmul(
                                pss[j][:, 0:512], w_sb[:, ff[j], ko, :],
                                h0[:, ko, :],
                                start=(ko == 0), stop=(ko == KO - 1))
                    for j in range(nfuse):
                        for ko in range(KO):
                            nc.tensor.matmul(
                                pss[j][:, 512:], w_sb[:, ff[j], ko, :],
                                h1[:, ko, :],
                                start=(ko == 0), stop=(ko == KO - 1))
                        _evac_chunk(nc, ff[j], pss[j], js, tsl, qk_sb, v_sb,
                                    cos_sb, sin_sb, sb_pools)
                    continue
                ps = qkps.tile([128, TSLAB], F32, name="ps")
                # halves INTERLEAVED per ko on purpose: consecutive
                # matmuls alternate psum banks, so each matmul's drain
                # overlaps the next one's fill (bank-contiguous halves
                # measured +130us -- the drain serializes against the
                # same-bank fill)
                for ko in range(KO):
                    nc.tensor.matmul(ps[:, 0:512], w_sb[:, f, ko, :],
                                     h0[:, ko, :],
                                     start=(ko == 0), stop=(ko == KO - 1))
                    nc.tensor.matmul(ps[:, 512:], w_sb[:, f, ko, :],
                                     h1[:, ko, :],
                                     start=(ko == 0), stop=(ko == KO - 1))
                _evac_chunk(nc, f, ps, js, tsl, qk_sb, v_sb, cos_sb, sin_sb,
                            sb_pools)


def _evac_chunk(nc, f, ps, js, tsl, qk_sb, v_sb, cos_sb, sin_sb, sb_pools):
    """Evacuate one qkv psum chunk: RoPE for q/k, v^T transpose for v."""
    rawpool, shufpool, vtpool = (sb_pools["raw"], sb_pools["shuf"],
                                 sb_pools["vt"])
    if f < 8:
        # q/k chunk: RoPE
        raw = rawpool.tile([128, TSLAB], BF16, name="raw")
        nc.scalar.copy(raw, ps)
        shuf = shufpool.tile([128, TSLAB], BF16, name="shuf")
        # rotate-half pair swap across adjacent partitions;
        # issued on the scalar queue right after the evac
        nc.scalar.dma_start(shuf[0:127:2, :], raw[1:128:2, :])
        nc.scalar.dma_start(shuf[1:128:2, :], raw[0:127:2, :])
        # t1 = raw*cos in place (Pool); t2 = shuf*sin_signed
        # in place (DVE); sum into the resident qk chunk
        nc.gpsimd.tensor_tensor(raw, raw, cos_sb[:, tsl], ALU.mult)
        nc.vector.tensor_tensor(shuf, shuf, sin_sb[:, tsl], ALU.mult)
        nc.vector.tensor_tensor(qk_sb[:, f, tsl], raw, shuf, ALU.add)
    else:
        # v chunk: evacuate v^T, wide-transpose into v_sb
        fc = f - 8
        vt = vtpool.tile([128, TSLAB], BF16, name="vt")
        nc.scalar.copy(vt, ps)
        nc.sync.dma_start_transpose(
            v_sb[:, js * (TSLAB // 128):(js + 1) * (TSLAB // 128),
                 fc * 128:(fc + 1) * 128],
            vt)


def _phase_attn(nc, tc, b, qk_sb, v_sb, y_sb, ones_c_sb, ones_bc_sb,
                tri_sb, sb_pools, pending, dma_thunks=()):
    """Causal attention per head, transposed orientation.
    scores^T [k, q] -> exp (single merged ACT instr) -> tri-mask (DVE) ->
    l (ones matmul, halves col-tiled to PE quadrants 0/1 so they stream
    concurrently), y^T = v_nat.T @ p^T; normalization via Pool partition
    broadcast.  l_ps is a single-bank [128, 512] tile: half A
    accumulates on partition 0, half B on partition 32."""
    ppool, nfpool, nbpool, bcpool = (sb_pools["p"], sb_pools["nf"],
                                     sb_pools["nb"], sb_pools["bc"])

    def jq_tail_p1(ctx):
        """End-of-q-tile part 1: finish evacuating y, 1/l for both halves
        in one base-0 DVE op (reciprocal_approx_fast and
        partition_broadcast both misbehave on base-partition-32 APs on
        hw; rows 1..31 are don't-care lanes), kick off the row-32 -> 0
        SBUF DMA and half-A broadcast.  The consumers run in part 2, a
        few blocks later, so the ~2us DMA latency never head-of-line
        blocks the DVE/PE queues."""
        h, q0, y_ps, l_ps = ctx
        nc.vector.tensor_copy(y_sb[:, h, q0 + 512:q0 + QT], y_ps[:, 512:])
        linv = nfpool.tile([33, 512], F32, name="linv")
        nc.vector.reciprocal_approx_fast(linv, l_ps[0:33, :])
        linv_bf = nbpool.tile([33, 512], BF16, name="linv_bf")
        nc.vector.tensor_copy(linv_bf, linv)
        bcA = bcpool.tile([128, 512], BF16, name="bc_sb")
        nc.gpsimd.partition_broadcast(bcA, linv_bf[0:1, :])
        return (h, q0, l_ps, linv_bf, bcA)

    def jq_tail_p2(ctx2):
        """End-of-q-tile part 2: half-B broadcast via a K=1 PE matmul
        from partition 32 (ones row-group 1) into the now-dead l_ps
        bank, then both normalizes."""
        h, q0, l_ps, linv_bf, bcA = ctx2
        nc.tensor.matmul(l_ps[:, :], ones_bc_sb[32:33, :],
                         linv_bf[32:33, :], start=True, stop=True,
                         tile_position=(32, 0))
        bcB = bcpool.tile([128, 512], BF16, name="bc_sb")
        nc.vector.tensor_copy(bcB, l_ps)
        nc.vector.tensor_tensor(y_sb[:, h, q0:q0 + 512],
                                y_sb[:, h, q0:q0 + 512], bcA, ALU.mult)
        nc.vector.tensor_tensor(y_sb[:, h, q0 + 512:q0 + QT],
                                y_sb[:, h, q0 + 512:q0 + QT], bcB, ALU.mult)

    with (
        tc.tile_pool(name="sps", bufs=2, space="PSUM") as sps,
        tc.tile_pool(name="yps", bufs=1, space="PSUM") as yps,
        tc.tile_pool(name="lps", bufs=2, space="PSUM") as lps,
    ):
        tiles = {}
        tails = []  # [countdown, ctx2] deferred tail-part-2 work
        thunks = list(dma_thunks)
        state = {"n": 0}

        def drain(pb):
            """Emit l/pv for an already-emitted block; on half-A/q-tile
            completion, emit the deferred evacuation work."""
            state["n"] += 1
            if thunks and state["n"] in (12, 22, 32):
                thunks.pop(0)()
            for t in tails:
                t[0] -= 1
            while tails and tails[0][0] <= 0:
                jq_tail_p2(tails.pop(0)[1])
            h, jq, kb, p_sb, qoff, boff = pb
            q0 = jq * QT
            nkb = NKBT * (jq + 1)
            last_a = min(nkb - 1, NKBT * jq + 3)
            if kb == 0:
                tiles[(h, jq)] = (yps.tile([128, QT], F32, name="y_ps"),
                                  lps.tile([128, 512], F32, name="l_ps"))
            y_ps, l_ps = tiles[(h, jq)]
            _emit_l_pv(nc, v_sb, ones_c_sb, h, l_ps, y_ps, last_a, nkb,
                       p_sb, kb, qoff, boff)
            if kb == last_a and last_a != nkb - 1:
                # half A complete: evacuate it early (frees the bank)
                nc.vector.tensor_copy(y_sb[:, h, q0:q0 + 512],
                                      y_ps[:, 0:512])
            if kb == nkb - 1:
                tails.append([3, jq_tail_p1((h, q0, y_ps, l_ps))])

        # flattened (h, jq, kb) stream with a skew that carries across
        # q-tile and head boundaries: scores(i+1..) issue while exp(i)
        # runs on ACT, then l/pv(i) consume p(i).  The first `pending`
        # blocks were pre-emitted inside the qkv tail, so the stream
        # enters attention with the exps already in flight.
        blocks = [(h, jq, kb)
                  for h in range(H_LOC)
                  for jq in range(NQT)
                  for kb in range(NKBT * (jq + 1))]

        def alloc_s():
            return sps.tile([128, QT], F32, name="s_ps")

        def p_alloc():
            return ppool.tile([128, QT], BF16, name="p_sb")

        queue = list(pending)
        for blk in blocks[len(pending):]:
            queue.append(_attn_block(nc, alloc_s, p_alloc, qk_sb, tri_sb, blk))
            if len(queue) > 1:
                drain(queue.pop(0))
        while queue:
            drain(queue.pop(0))
        return [t[1] for t in tails], jq_tail_p2


def _attn_block(nc, alloc_s, p_alloc, qk_sb, tri_sb, blk):
    """Emit scores + exp + causal mask for one (h, jq, kb) block."""
    h, jq, kb = blk
    q0 = jq * QT
    qt = qk_sb[:, h, :]
    kt = qk_sb[:, 4 + h, :]
    s_diag = kb - NKBT * jq
    qoff = 128 * s_diag if s_diag > 0 else 0
    ksl = slice(kb * 128, (kb + 1) * 128)
    boff = max(0, qoff - 512)
    s_ps = alloc_s()
    if qoff < 512:
        nc.tensor.matmul(
            s_ps[:, qoff:512], kt[:, ksl],
            qt[:, q0 + qoff:q0 + 512], start=True, stop=True)
    nc.tensor.matmul(
        s_ps[:, 512 + boff:], kt[:, ksl],
        qt[:, q0 + 512 + boff:q0 + QT], start=True, stop=True)
    p_sb = p_alloc()
    nc.scalar.activation(p_sb[:, qoff:], s_ps[:, qoff:],
                         AF.Exp, scale=SCALE)
    if s_diag >= 0:
        # causal: zero p where q < k in the diagonal block
        nc.vector.tensor_tensor(
            p_sb[:, qoff:qoff + 128], p_sb[:, qoff:qoff + 128],
            tri_sb, ALU.mult)
    return (h, jq, kb, p_sb, qoff, boff)


def _emit_l_pv(nc, v_sb, ones_c_sb, h, l_ps, y_ps, last_a, nkb, p_sb, kb,
               qoff, boff):
    """l += ones.T @ p ; y^T += v_nat.T @ p^T for one key block.
    Half A (q cols [0,512)) ends at last_a; half B at nkb-1.
    The two l matmuls are col-tiled to PE quadrants 0/1 (out partitions
    0 and 32 of a single 1-bank l tile) so they stream concurrently."""
    hsl = slice(h * 128, (h + 1) * 128)
    if qoff < 512:
        nc.tensor.matmul(l_ps[0:1, qoff:512], ones_c_sb, p_sb[:, qoff:512],
                         start=(kb == 0), stop=(kb == last_a),
                         tile_position=(0, 0))
    nc.tensor.matmul(l_ps[32:33, boff:512], ones_c_sb, p_sb[:, 512 + boff:],
                     start=(kb == 0), stop=(kb == nkb - 1),
                     tile_position=(0, 32))
    # one v lhsT load serves both halves
    if qoff < 512:
        nc.tensor.matmul(y_ps[:, qoff:512], v_sb[:, kb, hsl],
                         p_sb[:, qoff:512],
                         start=(kb == 0), stop=(kb == last_a))
    nc.tensor.matmul(y_ps[:, 512 + boff:], v_sb[:, kb, hsl],
                     p_sb[:, 512 + boff:],
                     start=(kb == 0), stop=(kb == nkb - 1))


def _phase_proj(nc, tc, b, wppool, w_proj, y_sb, out_t, sb_pools,
                tails=(), flush=None):
    """out^T[c, t] partial = Wp_loc.T @ y^T.  jt-outer/h-inner:
    consecutive matmuls accumulate into ONE psum bank (no per-MM bank
    switching) and each jt's evacuation overlaps the next jt's matmuls.
    wp streamed per c-chunk (prefetched 3 deep)."""
    NJT = T // 512
    opool = sb_pools["o"]
    wp4 = w_proj.rearrange("(h p) c -> p h c", p=128)
    with tc.tile_pool(name="opsum", bufs=7, space="PSUM") as opsum:
        wq = []
        for co in range(3):
            wt = wppool.tile([128, H_LOC, 128], BF16, name="wp_sb")
            nc.sync.dma_start(wt, wp4[:, :, co * 128:(co + 1) * 128])
            wq.append(wt)
        for co in range(C // 128):
            if co + 3 < C // 128:
                wt = wppool.tile([128, H_LOC, 128], BF16, name="wp_sb")
                nc.sync.dma_start(
                    wt, wp4[:, :, (co + 3) * 128:(co + 4) * 128])
                wq.append(wt)
            wp_sb = wq[co]
            csl = slice(co * 128, (co + 1) * 128)
            o_ps = [opsum.tile([128, 512], F32, name="o_ps") for _ in range(NJT)]
            for jt in range(NJT):
                for h in range(H_LOC):
                    if co == 0 and jt == 0 and h == H_LOC - 1 and tails:
                        # last attention group's deferred tail: emit it
                        # under the h<3 proj matmuls, before h3's reads
                        for t in tails:
                            flush(t)
                        tails = ()
                    nc.tensor.matmul(
                        o_ps[jt], wp_sb[:, h, :],
                        y_sb[:, h, jt * 512:(jt + 1) * 512],
                        start=(h == 0), stop=(h == H_LOC - 1))
                o_sb = opool.tile([128, 512], BF16, name="o_sb")
                # alternate ACT/DVE for psum evacuation; the write rides
                # the matching queue to halve issue latency
                if jt % 2 == 0:
                    nc.scalar.copy(o_sb, o_ps[jt])
                    nc.scalar.dma_start(
                        out_t[b, csl, jt * 512:(jt + 1) * 512], o_sb)
                else:
                    nc.vector.tensor_copy(o_sb, o_ps[jt])
                    nc.sync.dma_start(
                        out_t[b, csl, jt * 512:(jt + 1) * 512], o_sb)


def _get_nc():
    if "nc" not in _CACHED:
        _CACHED["nc"] = build_nc()
    return _CACHED["nc"]


def kernel(x, sin, cos, W_qkv, W_proj):
    x = np.asarray(x, dtype=np.float32)
    sin = np.asarray(sin, dtype=np.float32)
    cos = np.asarray(cos, dtype=np.float32)
    W_qkv = np.asarray(W_qkv, dtype=np.float32)
    W_proj = np.asarray(W_proj, dtype=np.float32)

    # rotate-half is a pure pair swap on chip; the sign lives in sin:
    # roped[2i] = raw[2i]cos - raw[2i+1]sin ; roped[2i+1] = raw[2i+1]cos
    # + raw[2i]sin  =>  sin row 2i negated.
    sin_tn = np.ascontiguousarray(sin[0, 0].T).copy()  # [HD, T]
    sin_tn[0::2, :] *= -1.0
    sin_t = sin_tn.astype(NPBF)
    cos_t = np.ascontiguousarray(cos[0, 0].T).astype(NPBF)
    ones_col = np.ones((128, 1), NPBF)
    ones_bc = np.ones((33, 128), NPBF)
    tri = np.triu(np.ones((128, 128), np.float32)).astype(NPBF)

    in_maps = []
    for g in range(BGROUPS):
        x_tg = np.ascontiguousarray(
            x[g * B_LOC:(g + 1) * B_LOC].transpose(0, 2, 1)
        ).astype(NPBF)  # [B_LOC, C, T]
        for s in range(HSHARDS):
            qcols = W_qkv[:, s * FQK:(s + 1) * FQK]
            kcols = W_qkv[:, C + s * FQK:C + (s + 1) * FQK]
            vcols = W_qkv[:, 2 * C + s * FV:2 * C + (s + 1) * FV]
            w_flat = np.concatenate([qcols, kcols, vcols], axis=1)
            # [C, 1536] -> [p, chunk, ko, f] with C = ko*128 + p
            w_qkv_loc = np.ascontiguousarray(
                w_flat.reshape(KO, 128, NCH, 128).transpose(1, 2, 0, 3)
            ).astype(NPBF)
            w_proj_loc = np.ascontiguousarray(
                W_proj[s * FV:(s + 1) * FV, :]).astype(NPBF)
            in_maps.append(
                {
                    "x_t": x_tg,
                    "w_qkv": w_qkv_loc,
                    "w_proj": w_proj_loc,
                    "sin_t": sin_t,
                    "cos_t": cos_t,
                    "ones_col": ones_col,
                    "ones_bc": ones_bc,
                    "tri": tri,
                }
            )

    trace = bool(int(os.environ.get("KERNEL_TRACE", "0")))
    if trace:
        _install_ntff_hook()
    nc = _get_nc()
    res = run_bass_kernel_spmd(
        nc, in_maps, core_ids=list(range(NCORES)), trace=trace
    )
    _CACHED["last_result"] = res

    out = np.zeros((B, T, C), dtype=np.float32)
    for g in range(BGROUPS):
        acc = np.zeros((B_LOC, C, T), dtype=np.float32)
        for s in range(HSHARDS):
            acc += res.results[g * HSHARDS + s]["out_t"].astype(np.float32)
        out[g * B_LOC:(g + 1) * B_LOC] = acc.transpose(0, 2, 1)
    return out

